# revision 33
# baseline (speedup 1.0000x reference)
"""Fused MHA-with-RoPE kernel for one TRN2 chip (8 NeuronCores).

Sharding: core c handles batch b = c//2 and head-group g = c%2 (8 of 16
heads).  All matmuls run in bf16 (1 cycle/row on the PE, same as f32r,
but half the DMA/SBUF), accumulating in fp32 PSUM.

  phase 1: QKV projections + RoPE.  k and v are written DIRECTLY into
           SBUF-resident tiles (no DRAM round-trip); q is spilled to DRAM
           per head and reloaded per (ib, h) query block.  Startup DMAs
           are choreographed across the sync/scalar/gpsimd queues so the
           first matmul starts ~15us in.
  phase 2: causal attention, query-block (ib) outer / head inner, computed
           transposed (sT[j,i]) so no P transposes are needed.  The PE
           stream is software-pipelined 3 tiles deep ACROSS heads (scores
           for tile t+3 issue before den/av of tile t) so the in-order PE
           never waits on the mask+exp chain.  Diagonal tiles are trimmed
           to their live columns.  The softmax denominator rides a
           [128,128] ones-matmul whose PSUM output IS the partition
           broadcast; normalization = fast DVE reciprocal + one multiply.
  phase 3: interleaved per-ib: after the 8 heads of a query block finish,
           the 512-row output-projection chunk is emitted and its
           pair-ReduceScatter issued immediately, overlapping the next
           query block's attention (only the last RS is exposed).
"""

import math
import os
import sys
import types
from contextlib import ExitStack

import numpy as np
import ml_dtypes

import concourse.bass as bass
import concourse.tile as tile
from concourse import bacc, mybir
from concourse.bass_utils import run_bass_kernel_spmd

# ---------------------------------------------------------------- constants
B, S, D = 4, 2048, 2048
H, HD = 16, 128
GROUPS = 2            # head groups (cores per batch)
HLOC = H // GROUPS    # heads per core = 8
E = HLOC * HD         # local qkv width = 1024
N_CORES = 8
CORE_IDS = list(range(N_CORES))
SCALE = 1.0 / math.sqrt(HD)
NEG = -1.0e30
ROPE_BASE = 10000.0

F32 = mybir.dt.float32
BF16 = mybir.dt.bfloat16
FP8 = mybir.dt.float8e4
BF = ml_dtypes.bfloat16
E4 = ml_dtypes.float8_e4m3fn

_cache = {}


def _register_ntff_hook():
    """trn_boot can't register the NTFF profile hook (antenv.axon_hooks is
    missing from this image); recreate it so BASS_TRACE=1 profiling works."""
    if "antenv.axon_hooks" in sys.modules:
        return
    try:
        from trn_agent_boot.trn_boot import _ntff_profile_via_ctypes

        holder = {"h": _ntff_profile_via_ctypes("/opt/axon/libaxon_pjrt.so")}
        mod = types.ModuleType("antenv.axon_hooks")
        mod.get_axon_ntff_profile_hook = lambda: holder["h"]
        mod.set_axon_ntff_profile_hook = lambda h: holder.__setitem__("h", h)
        sys.modules["antenv.axon_hooks"] = mod
    except Exception:
        pass


def _host_tables():
    inv_freq = 1.0 / (ROPE_BASE ** (np.arange(0, HD, 2, dtype=np.float64) / HD))
    pos = np.arange(S, dtype=np.float64)
    freqs = pos[:, None] * inv_freq[None, :]
    emb = np.concatenate([freqs, freqs], axis=-1)        # [S, HD]
    cosT = np.ascontiguousarray(np.cos(emb).T.astype(np.float32))  # [HD, S]
    sinT = np.ascontiguousarray(np.sin(emb).T.astype(np.float32))
    sinF = sinT.copy()
    sinF[: HD // 2] *= -1.0                              # fold rotate_half sign
    return cosT, sinF


def _host_masks():
    # masks[j_local, o, i_local]: 0 if i_local >= o*128 + j_local else NEG
    m = np.empty((128, 4, 512), np.float32)
    jj = np.arange(128)[:, None]
    ii = np.arange(512)[None, :]
    for o in range(4):
        m[:, o, :] = np.where(ii >= o * 128 + jj, 0.0, NEG)
    return m


def _build_nc():
    nc = bacc.Bacc("TRN2", target_bir_lowering=False, debug=False,
                   num_devices=N_CORES)

    # host-pre-tiled inputs: partition-contiguous DMA layouts, all bf16
    xs_e = nc.dram_tensor("xs", [4, 128, 16, 512], BF16, kind="ExternalInput")
    wq_e = nc.dram_tensor("wq", [HLOC, 128, 16, 128], BF16,
                          kind="ExternalInput")
    wk_e = nc.dram_tensor("wk", [HLOC, 128, 16, 128], BF16,
                          kind="ExternalInput")
    wv_e = nc.dram_tensor("wv", [4, 128, 16, 256], BF16, kind="ExternalInput")
    wo_e = nc.dram_tensor("wo", [128, HLOC, D], BF16, kind="ExternalInput")
    # out[ib, row, :] = RS result rows ib*512 + g*256 .. +256 (bf16;
    # host upcasts to f32)
    out_e = nc.dram_tensor("out", [4, 512 // GROUPS, D], BF16,
                           kind="ExternalOutput")

    cosT_d = nc.inline_tensor(_host_tables()[0], name="cosT")
    sinF_d = nc.inline_tensor(_host_tables()[1], name="sinF")
    masks_d = nc.inline_tensor(_host_masks(), name="masks")
    ones_col_d = nc.inline_tensor(np.ones((128, 128), np.float32), name="ones_col")

    HF = HD // 2

    with tile.TileContext(nc) as tc, ExitStack() as ctx:
        dram = ctx.enter_context(tc.tile_pool(name="dram", bufs=1, space="DRAM"))
        qh_d = [dram.tile([HD, S], BF16, name=f"qh_d{h}") for h in range(HLOC)]
        part_d = [dram.tile([512, D], BF16, name=f"part_d{c}")
                  for c in range(4)]
        rs_d = [dram.tile([512 // GROUPS, D], BF16, name=f"rs_d{c}")
                for c in range(4)]
        # tables go on the (otherwise idle) scalar DGE queue so the first
        # weight tile isn't queued behind 3 MB of constants on gpsimd
        consts = ctx.enter_context(tc.tile_pool(name="consts", bufs=1))
        masks_sb = consts.tile([128, 4, 512], F32)
        ones_f32 = consts.tile([128, 128], F32)
        ones_bf = consts.tile([128, 128], BF16)

        # small phase-2 pools hoisted so q/pT prefetch crosses the phase
        # boundary without anti-deps on phase-1 teardown
        qt_pool = ctx.enter_context(tc.tile_pool(name="qt", bufs=3))
        pT_pool = ctx.enter_context(tc.tile_pool(name="pT", bufs=6))

        # SBUF-resident k/v for all heads (filled directly by phase 1)
        kv_pool = ctx.enter_context(tc.tile_pool(name="kv", bufs=1))
        k_all = kv_pool.tile([128, HLOC, S], BF16)        # [hd, h, key]
        v_all = kv_pool.tile([128, 16, HLOC, HD], BF16)   # [key%128, st, h, hd]

        # ---------------- phase 1: projections ----------------
        with tc.tile_pool(name="xT", bufs=1) as xT_pool, \
             tc.tile_pool(name="tabs", bufs=1) as tabs, \
             tc.tile_pool(name="wqk", bufs=2) as wqk_pool, \
             tc.tile_pool(name="wv", bufs=2) as wv_pool, \
             tc.tile_pool(name="rope_wk", bufs=3) as rwk, \
             tc.tile_pool(name="rot_out", bufs=3) as rout, \
             tc.tile_pool(name="ps1", bufs=4, space="PSUM") as ps1, \
             tc.tile_pool(name="psv", bufs=2, space="PSUM") as psv:
            cos_sb = tabs.tile([HD, S], F32)
            sinF_sb = tabs.tile([HD, S], F32)

            # startup choreography: sync queue carries x0,x2; the scalar
            # queue interleaves x1,x3 with per-slice cos/sin so each table
            # slice lands just before the rope that consumes it; masks/ones
            # (phase-2 consts) trail everything
            xs = [xT_pool.tile([128, 16, 512], BF16, name=f"xs{sb}")
                  for sb in range(4)]
            nc.sync.dma_start(out=xs[0][:, 0:8], in_=xs_e[0, :, 0:8])
            nc.scalar.dma_start(out=xs[0][:, 8:16], in_=xs_e[0, :, 8:16])
            nc.sync.dma_start(out=xs[1][:, 0:8], in_=xs_e[1, :, 0:8])
            nc.scalar.dma_start(out=xs[1][:, 8:16], in_=xs_e[1, :, 8:16])
            nc.sync.dma_start(out=xs[2][:], in_=xs_e[2])
            for sb in range(4):
                nc.scalar.dma_start(out=cos_sb[:, bass.ts(sb, 512)],
                                    in_=cosT_d[:, bass.ts(sb, 512)])
                nc.scalar.dma_start(out=sinF_sb[:, bass.ts(sb, 512)],
                                    in_=sinF_d[:, bass.ts(sb, 512)])
                if sb == 0:
                    nc.sync.dma_start(out=xs[3][:], in_=xs_e[3])
            nc.scalar.dma_start(out=masks_sb[:], in_=masks_d[:])
            nc.scalar.dma_start(out=ones_f32[:], in_=ones_col_d[:])
            nc.scalar.copy(ones_bf[:], ones_f32[:])

            # q/k projections + RoPE.  k goes straight into k_all; q is
            # spilled transposed per head [HD, S].
            for w_e, is_q, pname in ((wq_e, True, "q"), (wk_e, False, "k")):
                for m in range(HLOC):
                    w_sb = wqk_pool.tile([128, 16, 128], BF16,
                                         name=f"w{pname}{m}", tag="w")
                    nc.gpsimd.dma_start(out=w_sb[:], in_=w_e[m])
                    for sb in range(4):
                        ps = ps1.tile([128, 512], F32, name="ps_qk",
                                      tag="ps_qk")
                        for dt_ in range(16):
                            nc.tensor.matmul(
                                ps[:], w_sb[:, dt_, :], xs[sb][:, dt_, :],
                                start=(dt_ == 0), stop=(dt_ == 15))
                        c_sl = cos_sb[:, bass.ts(sb, 512)]
                        s_sl = sinF_sb[:, bass.ts(sb, 512)]
                        sw = rwk.tile([128, 512], F32, name="sw", tag="sw")
                        nc.scalar.copy(sw[0:HF, :], ps[HF:HD, :])
                        nc.scalar.copy(sw[HF:HD, :], ps[0:HF, :])
                        m1 = rwk.tile([128, 512], F32, name="m1", tag="m1")
                        nc.vector.tensor_mul(m1[:], ps[:], c_sl)
                        m2 = rwk.tile([128, 512], F32, name="m2", tag="m2")
                        nc.vector.tensor_mul(m2[:], sw[:], s_sl)
                        if is_q:
                            rot = rout.tile([128, 512], BF16, name="rot",
                                            tag="rot")
                            nc.vector.tensor_add(rot[:], m1[:], m2[:])
                            nc.gpsimd.dma_start(
                                out=qh_d[m][:, bass.ts(sb, 512)], in_=rot[:])
                        else:
                            nc.vector.tensor_add(
                                k_all[:, m, bass.ts(sb, 512)], m1[:], m2[:])

            # v projection, written straight into v_all
            for n in range(4):                     # 256-wide = 2 heads
                wv_sb = wv_pool.tile([128, 16, 256], BF16, name=f"wv{n}",
                                     tag="wv")
                nc.sync.dma_start(out=wv_sb[:], in_=wv_e[n])
                for st in range(16):
                    ps = psv.tile([128, 256], F32, name="ps_v", tag="ps_v")
                    for dt_ in range(16):
                        nc.tensor.matmul(
                            ps[:], xs[st // 4][:, dt_, bass.ts(st % 4, 128)],
                            wv_sb[:, dt_, :],
                            start=(dt_ == 0), stop=(dt_ == 15))
                    nc.scalar.copy(v_all[:, st, 2 * n, :], ps[:, 0:128])
                    nc.scalar.copy(v_all[:, st, 2 * n + 1, :], ps[:, 128:256])

        # ---------------- phase 2+3: attention + fused out-proj ----------
        with tc.tile_pool(name="wo", bufs=1) as wo_pool, \
             tc.tile_pool(name="nrm", bufs=2) as nrm_pool, \
             tc.tile_pool(name="avT", bufs=2) as avT_pool, \
             tc.tile_pool(name="po", bufs=3) as po_pool, \
             tc.tile_pool(name="ps2", bufs=4, space="PSUM") as ps2, \
             tc.tile_pool(name="psav", bufs=2, space="PSUM") as psav, \
             tc.tile_pool(name="psden", bufs=2, space="PSUM") as psden:
            wo_sb = wo_pool.tile([128, HLOC, D], BF16)
            nc.gpsimd.dma_start(out=wo_sb[:], in_=wo_e[:])

            q_tiles = {}

            def load_q(ib2, h2):
                t = qt_pool.tile([128, 512], BF16, name="q_t", tag="qt")
                nc.sync.dma_start(out=t[:], in_=qh_d[h2][:, bass.ts(ib2, 512)])
                q_tiles[(ib2, h2)] = t

            load_q(0, 0)
            for ib in range(4):
                nj = 4 * (ib + 1)
                avT_cur = avT_pool.tile([128, HLOC, 512], BF16, name="avT",
                                        tag="avT")
                # per-head accumulators, created lazily at jt==0
                den_of = {}
                av_of = {}

                def den_av(h, jt, pT):
                    # den uses a [128,128] ones stationary so every PSUM
                    # partition holds the full denominator row — the
                    # partition broadcast comes for free out of the PE.
                    # Diagonal tiles only touch live columns [c0:512); the
                    # skipped columns hold p=0 so the partial accumulate is
                    # exact.
                    c0 = max(jt - 4 * ib, 0) * 128
                    nc.tensor.matmul(den_of[h][:, c0:], ones_bf[:],
                                     pT[:, c0:],
                                     start=(jt == 0), stop=(jt == nj - 1),
                                     skip_group_check=(c0 > 0))
                    nc.tensor.matmul(av_of[h][:, c0:], v_all[:, jt, h, :],
                                     pT[:, c0:],
                                     start=(jt == 0), stop=(jt == nj - 1),
                                     skip_group_check=(c0 > 0))
                    if jt == nj - 1:
                        rden = nrm_pool.tile([128, 512], F32, name="rden",
                                             tag="rden")
                        nc.vector.reciprocal_approx_fast(out=rden[:],
                                                         in_=den_of[h][:])
                        nc.vector.tensor_mul(avT_cur[:, h, :], av_of[h][:],
                                             rden[:])

                # software pipeline, depth 2, carried ACROSS heads: s(t+2)
                # is issued before den/av(t) so the in-order PE never waits
                # on the mask+exp chain (≈1.35us on diagonal tiles)
                pending = []
                for h in range(HLOC):
                    q_t = q_tiles.pop((ib, h))
                    if h + 1 < HLOC:
                        load_q(ib, h + 1)
                    elif ib + 1 < 4:
                        load_q(ib + 1, 0)
                    den_of[h] = psden.tile([128, 512], F32, name="den",
                                           tag="den")
                    av_of[h] = psav.tile([128, 512], F32, name="av", tag="av")
                    for jt in range(nj):
                        o_diag = jt - 4 * ib
                        c0 = max(o_diag, 0) * 128
                        s_ps = ps2.tile([128, 512], F32, name="s_ps",
                                        tag="s_ps")
                        nc.tensor.matmul(s_ps[:, c0:],
                                         k_all[:, h, bass.ts(jt, 128)],
                                         q_t[:, c0:], start=True, stop=True)
                        if len(pending) >= 3:
                            den_av(*pending.pop(0))
                        if o_diag >= 0:
                            nc.vector.tensor_add(s_ps[:, c0:], s_ps[:, c0:],
                                                 masks_sb[:, o_diag, c0:])
                        pT = pT_pool.tile([128, 512], BF16, name="pT",
                                          tag="pT")
                        nc.scalar.activation(
                            pT[:, c0:], s_ps[:, c0:],
                            mybir.ActivationFunctionType.Exp, scale=SCALE)
                        pending.append((h, jt, pT))
                for item in pending:
                    den_av(*item)
                pending = []

                # out-projection chunk for this 512-row query block; the
                # pair-RS is issued right behind it and runs on the CC core
                # while the next query block computes.  Nothing else lives
                # on the gpsimd queue, so the RS blocking it is harmless.
                for i4 in range(4):
                    for eb in range(4):
                        ps = psav.tile([128, 512], F32, name="ps_o", tag="av")
                        for hh in range(HLOC):
                            nc.tensor.matmul(
                                ps[:], avT_cur[:, hh, bass.ts(i4, 128)],
                                wo_sb[:, hh, bass.ts(eb, 512)],
                                start=(hh == 0), stop=(hh == HLOC - 1))
                        po = po_pool.tile([128, 512], BF16, name="po",
                                          tag="po")
                        nc.vector.tensor_copy(po[:], ps[:])
                        nc.sync.dma_start(
                            out=part_d[ib][bass.ts(i4, 128),
                                           bass.ts(eb, 512)],
                            in_=po[:])
                nc.gpsimd.collective_compute(
                    "ReduceScatter",
                    mybir.AluOpType.add,
                    replica_groups=[[0, 1], [2, 3], [4, 5], [6, 7]],
                    ins=[part_d[ib][:]],
                    outs=[rs_d[ib][:]],
                )
                nc.gpsimd.dma_start(out=out_e[ib], in_=rs_d[ib][:])

    nc.compile()
    return nc


def kernel(x, Wq, Wk, Wv, Wo):
    _register_ntff_hook()
    if "nc" not in _cache:
        _cache["nc"] = _build_nc()
    nc = _cache["nc"]

    in_maps = []
    for c in CORE_IDS:
        b, g = c // GROUPS, c % GROUPS
        sl = slice(g * E, (g + 1) * E)
        xT = np.ascontiguousarray(x[b].T)                       # [D, S]
        in_maps.append({
            "xs": np.ascontiguousarray(
                xT.reshape(16, 128, 4, 512).transpose(2, 1, 0, 3)).astype(BF),
            "wq": np.ascontiguousarray(
                Wq[sl, :].T.reshape(16, 128, HLOC, 128)
                .transpose(2, 1, 0, 3)).astype(BF),
            "wk": np.ascontiguousarray(
                Wk[sl, :].T.reshape(16, 128, HLOC, 128)
                .transpose(2, 1, 0, 3)).astype(BF),
            "wv": np.ascontiguousarray(
                Wv[sl, :].T.reshape(16, 128, 4, 256)
                .transpose(2, 1, 0, 3)).astype(BF),
            "wo": np.ascontiguousarray(
                Wo[:, sl].T.reshape(HLOC, 128, D).transpose(1, 0, 2)).astype(BF),
        })

    trace = bool(os.environ.get("BASS_TRACE"))
    res = run_bass_kernel_spmd(nc, in_maps, CORE_IDS, trace=trace)
    kernel.last_exec_time_ns = res.exec_time_ns
    kernel.last_res = res

    out = np.empty((B, S, D), np.float32)
    half = 512 // GROUPS
    for c in CORE_IDS:
        b, g = c // GROUPS, c % GROUPS
        r = np.asarray(res.results[c]["out"]).astype(np.float32)  # [4, 256, D]
        for ch in range(4):
            lo = ch * 512 + g * half
            out[b, lo:lo + half, :] = r[ch]
    return out


kernel.last_exec_time_ns = None
